# revision 20
# baseline (speedup 1.0000x reference)
"""2D DCT-II (ortho) on (32, 3, 512, 512) fp32, data-parallel across 8 TRN2 NeuronCores.

The DCT along an axis is a matmul with the constant 512x512 DCT matrix D:
    out = D @ X @ D.T
Structure (per 512x512 image, all matmuls float32r, fp32 PSUM accumulation):
  0. W-axis even/odd fold (DVE, reversed-AP second operand):
       EW[n,w'] = X[n,w'] + X[n,511-w'],  OW[n,w'] = X[n,w'] - X[n,511-w']
     (uses D[k, 511-w] = (-1)^k D[k, w], halving pass B's contraction)
  1. Pass A (H-DCT): P1e = EW.T @ D.T, P1o = OW.T @ D.T  via lhsT=EW/OW
     chunks (data stationary), rhs = D.T. 16 matmuls N=512.
  2. Pass B (W-DCT): OUT[:, 2j] = P1e.T @ DeW.T, OUT[:, 2j+1] = P1o.T @ DoW.T
     with DeW[j,w'] = D[2j,w'], DoW[j,w'] = D[2j+1,w']. 16 matmuls N=256;
     the PSUM->SBUF copy interleaves the even/odd column blocks.
Engine budget per image: PE 32 MMs (~6us), DVE folds + pass-B interleave
copies, ACT pass-A copies + store dispatch, sync load dispatch.
"""
import os
import sys

for _p in ("/opt/trn_rl_repo", os.path.expanduser("~/.axon_site/_ro/trn_rl_repo")):
    if os.path.isdir(_p) and _p not in sys.path:
        sys.path.insert(0, _p)

import numpy as np
import concourse.bass as bass
import concourse.bacc as bacc
import concourse.mybir as mybir
import concourse.tile as tile
from concourse.bass_utils import run_bass_kernel_spmd

dt = mybir.dt

N = 512            # image height/width
H = N // 2         # 256, folded width
P = 128            # SBUF partitions
C = N // P         # 4 row-chunks per image
N_CORES = 8
B, CH = 32, 3      # full input batch/channels
IMGS = (B * CH) // N_CORES  # 12 images per core


def _dct_matrix() -> np.ndarray:
    n = np.arange(N, dtype=np.float64)
    k = n[:, None]
    D = np.cos(np.pi * (2.0 * n[None, :] + 1.0) * k / (2.0 * N))
    D[0] *= np.sqrt(1.0 / N)
    D[1:] *= np.sqrt(2.0 / N)
    return D


def _consts() -> tuple[np.ndarray, np.ndarray]:
    D = _dct_matrix()
    dct_t = np.ascontiguousarray(D.T.astype(np.float32))            # [n, k]
    de_t = np.ascontiguousarray(D[0::2, :H].T.astype(np.float32))   # [256, 256]
    do_t = np.ascontiguousarray(D[1::2, :H].T.astype(np.float32))   # [256, 256]
    deo = np.concatenate([de_t, do_t], axis=0)                      # [512, 256]
    return dct_t, deo


def _build_nc() -> bacc.Bacc:
    nc = bacc.Bacc("TRN2", target_bir_lowering=False, debug=False, num_devices=N_CORES)
    inp = nc.dram_tensor("inp", [IMGS, N, N], dt.float32r, kind="ExternalInput")
    out = nc.dram_tensor("out", [IMGS, N, N], dt.float32, kind="ExternalOutput")
    dct_t = nc.dram_tensor("dct_t", [N, N], dt.float32r, kind="ExternalInput")
    deo_t = nc.dram_tensor("deo_t", [N, H], dt.float32r, kind="ExternalInput")

    f32r = dt.float32r
    f32 = dt.float32

    with tile.TileContext(nc) as tc:
        with (
            tc.tile_pool(name="const", bufs=1) as const_pool,
            tc.tile_pool(name="xin", bufs=4) as xin_pool,
            tc.tile_pool(name="eo", bufs=2) as eo_pool,
            tc.tile_pool(name="mid", bufs=2) as mid_pool,
            tc.tile_pool(name="res", bufs=2) as res_pool,
            tc.tile_pool(name="ps", bufs=3, space="PSUM") as psa_pool,
            tc.tile_pool(name="psb", bufs=2, space="PSUM") as psb_pool,
        ):
            # D.T resident in SBUF: dt_sb[p, 512*c + f] = D.T[128*c + p, f]
            dt_sb = const_pool.tile([P, C * N], f32r)
            nc.scalar.dma_start(
                dt_sb[:].rearrange("p (c f) -> p c f", c=C),
                dct_t.ap().rearrange("(c p) f -> p c f", p=P),
            )
            # deo_sb[p, 256*q + j] = deo[128*q + p, j]; q=0,1 even, q=2,3 odd
            deo_sb = const_pool.tile([P, C * H], f32r)
            nc.scalar.dma_start(
                deo_sb[:].rearrange("p (q j) -> p q j", q=C),
                deo_t.ap().rearrange("(q p) j -> p q j", p=P),
            )

            # PE warmup during the initial DMA ramp: ~10 dummy matmuls flip the
            # HAM clock gate to 8/8 before the first real matmul arrives.
            scr_f = const_pool.tile([P, N + P], f32)
            nc.gpsimd.memset(scr_f[:], 0.0)
            scr = const_pool.tile([P, N + P], f32r)
            nc.vector.tensor_copy(scr[:], scr_f[:])
            ps_w = psb_pool.tile([P, N], f32, tag="psB")
            for _ in range(8):
                nc.tensor.matmul(
                    ps_w[:], scr[:, N : N + P], scr[:, :N], start=True, stop=True
                )

            for i in range(IMGS):
                # x_sb[p, 512*c + w] = X[128*c + p, w]
                x_sb = xin_pool.tile([P, C * N], f32r, tag="x")
                nhalf = 4 if i == 0 else 1  # finer pipelining for the first image
                for hh in range(nhalf):
                    cs, ce = hh * C // nhalf, (hh + 1) * C // nhalf
                    nc.sync.dma_start(
                        x_sb[:, N * cs : N * ce].rearrange("p (c f) -> p c f", c=ce - cs),
                        inp.ap()[i][P * cs : P * ce, :].rearrange("(c p) f -> p c f", p=P),
                    )

                # W fold; separate EW/OW tiles so pass-A windows only wait
                # on their own parity's fold. Adds first (pass A eats EW first).
                # Image 0's EW lives in per-half tiles so its first matmuls
                # need only the first half-load + one fold op.
                if i == 0:
                    ew_tiles = [
                        eo_pool.tile([P, H], f32r, tag=f"ew0_{q}", name=f"ew0_{q}")
                        for q in range(C)
                    ]
                else:
                    ew_tiles = [
                        eo_pool.tile([P, C * H], f32r, tag="ew", name=f"ew_{i}")
                    ]
                ow_sb = eo_pool.tile([P, C * H], f32r, tag="ow")

                def ew_slice(c, col, width=P):
                    if i == 0:
                        return ew_tiles[c][:, col : col + width]
                    return ew_tiles[0][:, c * H + col : c * H + col + width]

                xa = x_sb[:]
                nsub = 2 if i == 0 else 1  # subs chunk-pair-wise for image 0
                for par in range(2):  # 0: add -> EW, 1: sub -> OW
                    nops = nhalf if par == 0 else nsub
                    for hh in range(nops):
                        cs, ce = hh * C // nops, (hh + 1) * C // nops
                        nc_ = ce - cs
                        lo = bass.AP(
                            xa.tensor, xa.offset + N * cs,
                            [[xa.ap[0][0], P], [N, nc_], [1, H]],
                        )
                        hi_rev = bass.AP(
                            xa.tensor, xa.offset + N * cs + N - 1,
                            [[xa.ap[0][0], P], [N, nc_], [-1, H]],
                        )
                        if par == 0:
                            dst = (
                                ew_tiles[0][:, H * cs : H * ce]
                                if i != 0
                                else ew_tiles[hh][:]
                            ).rearrange("p (c j) -> p c j", c=nc_)
                            nc.vector.tensor_add(dst, lo, hi_rev)
                        else:
                            dst = ow_sb[:, H * cs : H * ce].rearrange(
                                "p (c j) -> p c j", c=nc_
                            )
                            nc.vector.tensor_sub(dst, lo, hi_rev)

                # pass A (H-DCT): t in {e0,e1,o0,o1}; t-pairs share a 2-bank psum
                p1_sb = mid_pool.tile([P, C * N], f32r, tag="p1")
                for tp in range(2):
                    ps = psa_pool.tile([P, 2 * N], f32, tag="psA")
                    for t2 in range(2):
                        for c in range(C):
                            if tp == 0:
                                lhsT = ew_slice(c, t2 * P)
                            else:
                                lhsT = ow_sb[:, t2 * P + H * c : t2 * P + H * c + P]
                            rhs = dt_sb[:, N * c : N * (c + 1)]
                            nc.tensor.matmul(
                                ps[:, N * t2 : N * (t2 + 1)], lhsT, rhs,
                                start=(c == 0), stop=(c == C - 1),
                            )
                    nc.scalar.copy(p1_sb[:, 2 * N * tp : 2 * N * (tp + 1)], ps[:])

                # pass B (W-DCT): k_h windows m, single-bank psums
                # two half-image result tiles so each store waits only its half
                o_half = [
                    res_pool.tile([P, 2 * N], f32, tag="o0", name=f"oh0_{i}"),
                    res_pool.tile([P, 2 * N], f32, tag="o1", name=f"oh1_{i}"),
                ]
                for m in range(C):
                    ps = psb_pool.tile([P, N], f32, tag="psB")
                    for half in range(2):  # 0: even k_w, 1: odd k_w
                        for c2 in range(2):
                            q = 2 * half + c2
                            lhsT = p1_sb[:, N * q + P * m : N * q + P * (m + 1)]
                            rhs = deo_sb[:, H * q : H * (q + 1)]
                            nc.tensor.matmul(
                                ps[:, H * half : H * (half + 1)], lhsT, rhs,
                                start=(c2 == 0), stop=(c2 == 1),
                            )
                    # interleave: o[p, 512*(m%2) + 2j + h] = ps[p, 256*h + j]
                    src = ps[:].rearrange("p (h j) -> p h j", h=2)
                    ob = o_half[m // 2][:]
                    dst = bass.AP(
                        ob.tensor, ob.offset + N * (m % 2),
                        [[ob.ap[0][0], P], [1, 2], [2, H]],
                    )
                    nc.vector.tensor_copy(dst, src)
                    if i == IMGS - 1:  # tail: store each window immediately
                        nc.scalar.dma_start(
                            out.ap()[i][P * m : P * (m + 1), :],
                            o_half[m // 2][:, N * (m % 2) : N * (m % 2 + 1)],
                        )
                    elif m % 2 == 1:  # store half-image once its windows landed
                        mp = m // 2
                        nc.scalar.dma_start(
                            out.ap()[i][2 * P * mp : 2 * P * (mp + 1), :].rearrange(
                                "(c p) f -> p c f", p=P
                            ),
                            o_half[mp][:].rearrange("p (c f) -> p c f", c=2),
                        )

    nc.compile()
    return nc


_NC_CACHE: bacc.Bacc | None = None


def _get_nc() -> bacc.Bacc:
    global _NC_CACHE
    if _NC_CACHE is None:
        _NC_CACHE = _build_nc()
    return _NC_CACHE


def run(inp: np.ndarray, **spmd_kwargs):
    """Shard, run on 8 cores, gather. Returns (output, BassKernelResults)."""
    x = np.asarray(inp, dtype=np.float32)
    assert x.shape == (B, CH, N, N), x.shape
    shards = x.reshape(N_CORES, IMGS, N, N)
    dct_t, deo = _consts()
    in_maps = [
        {"inp": np.ascontiguousarray(shards[c]), "dct_t": dct_t, "deo_t": deo}
        for c in range(N_CORES)
    ]
    res = run_bass_kernel_spmd(_get_nc(), in_maps, core_ids=list(range(N_CORES)), **spmd_kwargs)
    out = np.stack([res.results[c]["out"] for c in range(N_CORES)])
    return out.reshape(B, CH, N, N), res


def kernel(inp: np.ndarray) -> np.ndarray:
    out, _ = run(inp)
    return out
